# revision 1
# baseline (speedup 1.0000x reference)
"""Trainium2 Bass kernel for nn_LogisticRegressionPerStock.

Math:  h = sigmoid(einsum("bts,st->bs", x, W1) + b1);  out = h @ W2.T + b2
Shapes: x [1024, 24, 8192], W1 [8192, 24], W2 [8192, 8192].

Sharding: stock dim S is split across 8 cores (SLOC = 1024 each).
Core k computes h[:, sk] from x[:, :, sk] (DVE multiply-accumulate over T
with a host-prereplicated W1 broadcast tile, then sigmoid on ACT), PE-
transposes h chunks into hT [s-part, b-free] layout, then runs the GEMM
partial_k = h[:, sk] @ W2[:, sk].T as fp32r matmuls (full [B, S] output).
The host sums the 8 partials and adds b2.  No collectives needed.
"""

import sys

sys.path.insert(0, "/opt/trn_rl_repo")

import numpy as np

B, T, S = 1024, 24, 8192
NCORES = 8
SLOC = S // NCORES  # 1024 stocks per core
S_CHUNK = 512  # einsum free-dim chunk
P = 128

_compiled = {}


def _build_nc(b, t, sloc, s_out, s_chunk):
    import concourse.bass as bass
    import concourse.bacc as bacc
    import concourse.tile as tile
    from concourse import mybir

    f32 = mybir.dt.float32
    f32r = mybir.dt.float32r
    nb = b // P          # b tiles
    nsc = sloc // s_chunk  # einsum chunks
    njc = s_chunk // P   # 128-blocks per chunk (transposes)
    nk = sloc // P       # GEMM contraction tiles
    n_out = s_out // 512  # GEMM output chunks of 512

    nc = bacc.Bacc()
    x_d = nc.dram_tensor("x", [b, t, sloc], f32, kind="ExternalInput")
    # w1r: partition-replicated [128, 25, sloc]; rows 0..t-1 = W1.T slice,
    # row t = b1 slice (bias folded into the same broadcast tile).
    w1r_d = nc.dram_tensor("w1r", [P, t + 1, sloc], f32, kind="ExternalInput")
    w2t_d = nc.dram_tensor("w2t", [sloc, s_out], f32r, kind="ExternalInput")
    ident_d = nc.dram_tensor("ident", [P, P], f32, kind="ExternalInput")
    out_d = nc.dram_tensor("part", [b, s_out], f32, kind="ExternalOutput")

    with tile.TileContext(nc) as tc:
        with tc.tile_pool(name="persist", bufs=1) as pp:
            ident = pp.tile([P, P], f32)
            nc.sync.dma_start(ident[:], ident_d[:])
            hT = pp.tile([P, nk, b], f32r)  # hT[sp, k, b] = h[b, k*128+sp]

            # ---- Phase E: per-stock logistic regressions -> hT ----
            with (
                tc.tile_pool(name="xp", bufs=2) as xp,
                tc.tile_pool(name="w1p", bufs=1) as w1p,
                tc.tile_pool(name="ep", bufs=4) as ep,
                tc.tile_pool(name="et", bufs=4) as etp,
                tc.tile_pool(name="eps", bufs=4, space="PSUM") as epsp,
            ):
                for c in range(nsc):
                    w1bc = w1p.tile([P, t + 1, s_chunk], f32, tag="w1bc")
                    nc.sync.dma_start(
                        w1bc[:], w1r_d[:, :, c * s_chunk : (c + 1) * s_chunk]
                    )
                    for bt in range(nb):
                        xt = xp.tile([P, t, s_chunk], f32, tag="xt")
                        nc.sync.dma_start(
                            xt[:],
                            x_d[
                                bt * P : (bt + 1) * P,
                                :,
                                c * s_chunk : (c + 1) * s_chunk,
                            ],
                        )
                        acc = ep.tile([P, s_chunk], f32, tag="acc")
                        nc.vector.tensor_mul(acc[:], xt[:, 0, :], w1bc[:, 0, :])
                        for ti in range(1, t):
                            tmp = ep.tile([P, s_chunk], f32, tag="tmp")
                            nc.vector.tensor_mul(
                                tmp[:], xt[:, ti, :], w1bc[:, ti, :]
                            )
                            nc.vector.tensor_add(acc[:], acc[:], tmp[:])
                        nc.vector.tensor_add(acc[:], acc[:], w1bc[:, t, :])
                        hs = ep.tile([P, s_chunk], f32, tag="hs")
                        nc.scalar.activation(
                            hs[:], acc[:], mybir.ActivationFunctionType.Sigmoid
                        )
                        for j in range(njc):
                            ptile = epsp.tile([P, P], f32, tag="pt")
                            nc.tensor.transpose(
                                ptile[:], hs[:, j * P : (j + 1) * P], ident[:]
                            )
                            k = c * njc + j
                            nc.vector.tensor_copy(
                                hT[:, k, bt * P : (bt + 1) * P], ptile[:]
                            )

            # ---- Phase G: partial = h_local @ W2[:, sk].T  (fp32r) ----
            with (
                tc.tile_pool(name="w2p", bufs=2) as w2p,
                tc.tile_pool(name="op", bufs=4) as op,
                tc.tile_pool(name="gps", bufs=8, space="PSUM") as gpsp,
            ):
                for n in range(n_out):
                    w2tiles = []
                    for k in range(nk):
                        w2k = w2p.tile([P, 512], f32r, tag=f"w2_{k}")
                        nc.sync.dma_start(
                            w2k[:],
                            w2t_d[k * P : (k + 1) * P, n * 512 : (n + 1) * 512],
                        )
                        w2tiles.append(w2k)
                    for bt in range(nb):
                        ps = gpsp.tile([P, 512], f32, tag="ps")
                        for k in range(nk):
                            nc.tensor.matmul(
                                ps[:],
                                hT[:, k, bt * P : (bt + 1) * P],
                                w2tiles[k][:],
                                start=(k == 0),
                                stop=(k == nk - 1),
                            )
                        ot = op.tile([P, 512], f32, tag="ot")
                        nc.vector.tensor_copy(ot[:], ps[:])
                        nc.sync.dma_start(
                            out_d[bt * P : (bt + 1) * P, n * 512 : (n + 1) * 512],
                            ot[:],
                        )
    nc.finalize()
    return nc


def _get_nc():
    key = (B, T, SLOC, S, S_CHUNK)
    if key not in _compiled:
        _compiled[key] = _build_nc(B, T, SLOC, S, S_CHUNK)
    return _compiled[key]


def _host_prep(x, W1, b1, W2):
    W2T = np.ascontiguousarray(W2.T)  # [S_in, S_out]
    ident = np.eye(P, dtype=np.float32)
    in_maps = []
    for k in range(NCORES):
        sk = slice(k * SLOC, (k + 1) * SLOC)
        x_k = np.ascontiguousarray(x[:, :, sk])
        w1e = np.concatenate(
            [W1[sk].T, b1[sk][None, :]], axis=0
        )  # [T+1, SLOC]
        w1r = np.ascontiguousarray(
            np.broadcast_to(w1e[None], (P, T + 1, SLOC))
        ).astype(np.float32)
        w2t_k = W2T[sk]  # contiguous row-slice view [SLOC, S]
        in_maps.append(
            {"x": x_k, "w1r": w1r, "w2t": np.ascontiguousarray(w2t_k), "ident": ident}
        )
    return in_maps


def kernel(x, W1, b1, W2, b2):
    from concourse.bass_utils import run_bass_kernel_spmd

    nc = _get_nc()
    in_maps = _host_prep(
        np.asarray(x, dtype=np.float32),
        np.asarray(W1, dtype=np.float32),
        np.asarray(b1, dtype=np.float32),
        np.asarray(W2, dtype=np.float32),
    )
    res = run_bass_kernel_spmd(nc, in_maps, list(range(NCORES)))
    parts = [res.results[k]["part"] for k in range(NCORES)]
    out = parts[0].astype(np.float32)
    for p in parts[1:]:
        out += p
    out += np.asarray(b2, dtype=np.float32)[None, :]
    return out



# revision 6
# speedup vs baseline: 1.6567x; 1.6567x over previous
"""Trainium2 Bass kernel for nn_LogisticRegressionPerStock.

Math:  h = sigmoid(einsum("bts,st->bs", x, W1) + b1);  out = h @ W2.T + b2
Shapes: x [1024, 24, 8192], W1 [8192, 24], W2 [8192, 8192].

Sharding: stock dim S split across 8 cores (SLOC = 1024 each); each core
computes a full [B, S] partial of the final GEMM (contraction over its
local stocks); host sums the 8 bf16 partials and adds b2.

Per-core layout: x is host-transposed to xT [S, T, B] fp16 so 128 stocks
sit on SBUF partitions.  The per-stock einsum runs on the DVE as 24 fused
scalar_tensor_tensor ops (acc = x[:,t,:]*W1[:,t] + acc, per-partition
scalars, fp16 2x mode); bias+sigmoid fuse into one ACT op that writes
hT [s-part, b-free] bf16 — already the GEMM's stationary layout, so no
PE transposes.  The [B,S] GEMM runs in bf16 (1 cycle/row) with hT slices
stationary (4 consecutive 512-wide matmuls per weight load) accumulating
over the 8 local k-tiles in PSUM.  B is processed in 4 chunks of 256 so
the DVE einsum of chunk c+1 overlaps the PE GEMM of chunk c; W2 (bf16,
resident in SBUF) prefetches during the first chunk's einsum.
"""

import sys

sys.path.insert(0, "/opt/trn_rl_repo")

import numpy as np
import ml_dtypes

B, T, S = 1024, 24, 8192
NCORES = 8
SLOC = S // NCORES  # 1024 stocks per core
P = 128
NSB = SLOC // P  # 8 s-blocks = GEMM k-tiles per core
BC = 256  # b-chunk size for the E/G pipeline
NBC = B // BC  # 4 chunks
NG = 4  # output column groups per bt (2048 cols each)
GW = S // NG  # 2048
NPS = GW // 512  # 4 psum tiles per group

_compiled = {}


def _build_nc():
    import concourse.bass as bass
    import concourse.bacc as bacc
    import concourse.tile as tile
    from concourse import mybir

    f32 = mybir.dt.float32
    f16 = mybir.dt.float16
    bf16 = mybir.dt.bfloat16
    MULT = mybir.AluOpType.mult
    ADD = mybir.AluOpType.add

    nc = bacc.Bacc()
    xt_d = nc.dram_tensor("xt", [SLOC, T, B], f16, kind="ExternalInput")
    # w1e: [128, NSB, T+1]; [p, k, t] = W1[k*128+p, t], [p, k, T] = b1[k*128+p]
    w1_d = nc.dram_tensor("w1e", [P, NSB, T + 1], f32, kind="ExternalInput")
    # w2t: [128, NSB, S]; [p, k, n] = W2.T[k*128+p, n] for this core's rows
    w2_d = nc.dram_tensor("w2t", [P, NSB, S], bf16, kind="ExternalInput")
    out_d = nc.dram_tensor("part", [B, S], bf16, kind="ExternalOutput")

    with tile.TileContext(nc) as tc:
        with (
            tc.tile_pool(name="persist", bufs=1) as pp,
            tc.tile_pool(name="xp", bufs=3) as xp,
            tc.tile_pool(name="ep", bufs=2) as ep,
            tc.tile_pool(name="stp", bufs=3) as stp,
            tc.tile_pool(name="psp", bufs=8, space="PSUM") as psp,
        ):
            w1sb = pp.tile([P, NSB, T + 1], f32)
            nc.sync.dma_start(w1sb[:], w1_d[:, :, :])
            w2sb = pp.tile([P, NSB, S], bf16)
            hts = [pp.tile([P, NSB, BC], bf16, name=f"ht{c}") for c in range(NBC)]

            def phase_e(c):
                for sb in range(NSB):
                    xtile = xp.tile([P, T, BC], f16, tag="xt")
                    nc.sync.dma_start(
                        xtile[:],
                        xt_d[sb * P : (sb + 1) * P, :, c * BC : (c + 1) * BC],
                    )
                    # prefetch W2 column-groups behind the x stream: group 0
                    # after chunk 0's x, groups 1-3 interleaved into chunk 1
                    if c == 0 and sb == NSB - 1:
                        nc.sync.dma_start(w2sb[:, :, 0:GW], w2_d[:, :, 0:GW])
                    elif c == 1 and sb in (1, 3, 5):
                        g = sb // 2 + 1
                        nc.sync.dma_start(
                            w2sb[:, :, g * GW : (g + 1) * GW],
                            w2_d[:, :, g * GW : (g + 1) * GW],
                        )
                    prev = None
                    for t in range(T):
                        cur = ep.tile([P, BC], f16, tag="acc")
                        if t == 0:
                            nc.vector.tensor_scalar(
                                cur[:], xtile[:, 0, :], w1sb[:, sb, 0:1], None, MULT
                            )
                        else:
                            nc.vector.scalar_tensor_tensor(
                                cur[:],
                                xtile[:, t, :],
                                w1sb[:, sb, t : t + 1],
                                prev[:],
                                MULT,
                                ADD,
                            )
                        prev = cur
                    nc.scalar.activation(
                        hts[c][:, sb, :],
                        prev[:],
                        mybir.ActivationFunctionType.Sigmoid,
                        bias=w1sb[:, sb, T : T + 1],
                    )

            def phase_g(c):
                for bl in range(BC // P):
                    bt = c * (BC // P) + bl
                    for g in range(NG):
                        pss = [
                            psp.tile([P, 512], f32, tag="ps", name=f"ps{n}")
                            for n in range(NPS)
                        ]
                        for k in range(NSB):
                            lhsT = hts[c][:, k, bl * P : (bl + 1) * P]
                            for n in range(NPS):
                                nc.tensor.matmul(
                                    pss[n][:],
                                    lhsT,
                                    w2sb[:, k, g * GW + n * 512 : g * GW + (n + 1) * 512],
                                    start=(k == 0),
                                    stop=(k == NSB - 1),
                                )
                        stg = stp.tile([P, GW], bf16, tag="stg")
                        for n in range(NPS):
                            nc.scalar.activation(
                                stg[:, n * 512 : (n + 1) * 512],
                                pss[n][:],
                                mybir.ActivationFunctionType.Copy,
                            )
                        nc.sync.dma_start(
                            out_d[bt * P : (bt + 1) * P, g * GW : (g + 1) * GW],
                            stg[:],
                        )

            for c in range(NBC):
                phase_e(c)
                phase_g(c)
    nc.finalize()
    return nc


def _get_nc():
    if "nc" not in _compiled:
        _compiled["nc"] = _build_nc()
    return _compiled["nc"]


def _host_prep(x, W1, b1, W2):
    xt = np.ascontiguousarray(x.transpose(2, 1, 0)).astype(np.float16)  # [S,T,B]
    w1e = np.concatenate(
        [W1.astype(np.float32), b1.astype(np.float32)[:, None]], axis=1
    )  # [S, T+1]
    W2T = W2.T.astype(ml_dtypes.bfloat16)  # [S_in, S_out]
    in_maps = []
    for k in range(NCORES):
        sk = slice(k * SLOC, (k + 1) * SLOC)
        in_maps.append(
            {
                "xt": xt[sk],
                # [P, NSB, ...] to match the SBUF tile dim order
                "w1e": np.ascontiguousarray(
                    w1e[sk].reshape(NSB, P, T + 1).transpose(1, 0, 2)
                ),
                "w2t": np.ascontiguousarray(
                    W2T[sk].reshape(NSB, P, S).transpose(1, 0, 2)
                ),
            }
        )
    return in_maps


def kernel(x, W1, b1, W2, b2):
    from concourse.bass_utils import run_bass_kernel_spmd

    nc = _get_nc()
    in_maps = _host_prep(
        np.asarray(x, dtype=np.float32),
        np.asarray(W1, dtype=np.float32),
        np.asarray(b1, dtype=np.float32),
        np.asarray(W2, dtype=np.float32),
    )
    res = run_bass_kernel_spmd(nc, in_maps, list(range(NCORES)))
    out = np.zeros((B, S), dtype=np.float32)
    for k in range(NCORES):
        out += res.results[k]["part"].astype(np.float32)
    out += np.asarray(b2, dtype=np.float32)[None, :]
    return out


# revision 12
# speedup vs baseline: 2.4818x; 1.4980x over previous
"""Trainium2 Bass kernel for nn_LogisticRegressionPerStock.

Math:  h = sigmoid(einsum("bts,st->bs", x, W1) + b1);  out = h @ W2.T + b2
Shapes: x [1024, 24, 8192], W1 [8192, 24], W2 [8192, 8192].

Sharding: stock dim S split across 8 cores (SLOC = 1024 each); each core
computes a full [B, S] partial of the final GEMM (contraction over its
local stocks); host sums the 8 bf16 partials and adds b2.

Per-core layout: W1 is folded into x on the host (x' = x * W1, transposed
to [S, B, T] fp16) so 128 stocks sit on SBUF partitions, t is innermost,
and the per-stock einsum is a single DVE tensor_reduce (sum over t) per
s-block; bias+sigmoid fuse into one ACT op that writes hT [s-part,
b-free] bf16 — already the GEMM's stationary layout, so no PE
transposes.  The [B,S] GEMM runs in bf16 (1 cycle/row) with hT slices
stationary (4 consecutive 512-wide matmuls per weight load) accumulating
over the 8 local k-tiles in PSUM.  B is processed in 4 chunks of 256 so
the DVE reduce of chunk c+1 overlaps the PE GEMM of chunk c; W2 (bf16,
resident in SBUF) prefetches behind the x stream.
"""

import sys

sys.path.insert(0, "/opt/trn_rl_repo")

import numpy as np
import ml_dtypes

B, T, S = 1024, 24, 8192
NCORES = 8
SLOC = S // NCORES  # 1024 stocks per core
P = 128
NSB = SLOC // P  # 8 s-blocks = GEMM k-tiles per core
BC = 256  # b-chunk size for the E/G pipeline
NBC = B // BC  # 4 chunks
NG = 4  # output column groups per bt (2048 cols each)
GW = S // NG  # 2048
NPS = GW // 512  # 4 psum tiles per group

_compiled = {}


def _build_nc():
    import concourse.bass as bass
    import concourse.bacc as bacc
    import concourse.tile as tile
    from concourse import mybir

    f32 = mybir.dt.float32
    f16 = mybir.dt.float16
    bf16 = mybir.dt.bfloat16
    MULT = mybir.AluOpType.mult
    ADD = mybir.AluOpType.add

    nc = bacc.Bacc()
    # xw: [S_loc, B, T] fp16, xw[s, b, t] = x[b, t, s] * W1[s, t]
    xt_d = nc.dram_tensor("xw", [SLOC, B, T], f16, kind="ExternalInput")
    # b1e: [128, NSB]; [p, k] = b1[k*128+p]
    b1_d = nc.dram_tensor("b1e", [P, NSB], f32, kind="ExternalInput")
    # w2t: [128, NSB, S]; [p, k, n] = W2.T[k*128+p, n] for this core's rows
    w2_d = nc.dram_tensor("w2t", [P, NSB, S], bf16, kind="ExternalInput")
    out_d = nc.dram_tensor("part", [B, S], bf16, kind="ExternalOutput")

    with tile.TileContext(nc) as tc:
        with (
            tc.tile_pool(name="persist", bufs=1) as pp,
            tc.tile_pool(name="xp", bufs=3) as xp,
            tc.tile_pool(name="ep", bufs=2) as ep,
            tc.tile_pool(name="stp", bufs=3) as stp,
            tc.tile_pool(name="psp", bufs=8, space="PSUM") as psp,
        ):
            b1sb = pp.tile([P, NSB], f32, tag="b1sb")
            nc.sync.dma_start(b1sb[:], b1_d[:, :])
            w2sb = pp.tile([P, NSB, S], bf16, tag="w2sb")
            hts = [
                pp.tile([P, NSB, BC], bf16, name=f"ht{c}", tag=f"ht{c}")
                for c in range(NBC)
            ]

            def phase_e(c):
                for sb in range(NSB):
                    xtile = xp.tile([P, BC, T], f16, tag="xt")
                    nc.sync.dma_start(
                        xtile[:],
                        xt_d[sb * P : (sb + 1) * P, c * BC : (c + 1) * BC, :],
                    )
                    # W2 column-groups stream behind chunk 0's x tiles; the
                    # chunk-0 GEMM's group-g matmuls block on quarter g's
                    # arrival via subtile RAW deps, later chunks are free
                    if c == 0 and sb % 2 == 1:
                        g = sb // 2
                        nc.sync.dma_start(
                            w2sb[:, :, g * GW : (g + 1) * GW],
                            w2_d[:, :, g * GW : (g + 1) * GW],
                        )
                    acc = ep.tile([P, BC], f16, tag="acc")
                    with nc.allow_low_precision("fp16 sum of 24 terms, validated"):
                        nc.vector.tensor_reduce(
                            acc[:], xtile[:], mybir.AxisListType.X, ADD
                        )
                    nc.scalar.activation(
                        hts[c][:, sb, :],
                        acc[:],
                        mybir.ActivationFunctionType.Sigmoid,
                        bias=b1sb[:, sb : sb + 1],
                    )

            def phase_g(c):
                for bl in range(BC // P):
                    bt = c * (BC // P) + bl
                    for g in range(NG):
                        pss = [
                            psp.tile([P, 512], f32, tag="ps", name=f"ps{n}")
                            for n in range(NPS)
                        ]
                        for k in range(NSB):
                            lhsT = hts[c][:, k, bl * P : (bl + 1) * P]
                            for n in range(NPS):
                                nc.tensor.matmul(
                                    pss[n][:],
                                    lhsT,
                                    w2sb[:, k, g * GW + n * 512 : g * GW + (n + 1) * 512],
                                    start=(k == 0),
                                    stop=(k == NSB - 1),
                                )
                        stg = stp.tile([P, GW], bf16, tag="stg")
                        for n in range(NPS):
                            nc.scalar.activation(
                                stg[:, n * 512 : (n + 1) * 512],
                                pss[n][:],
                                mybir.ActivationFunctionType.Copy,
                            )
                        nc.sync.dma_start(
                            out_d[bt * P : (bt + 1) * P, g * GW : (g + 1) * GW],
                            stg[:],
                        )

            for c in range(NBC):
                phase_e(c)
                phase_g(c)
    nc.finalize()
    return nc


def _get_nc():
    if "nc" not in _compiled:
        _compiled["nc"] = _build_nc()
    return _compiled["nc"]


def _host_prep(x, W1, b1, W2):
    # xw[s, b, t] = x[b, t, s] * W1[s, t], fp16
    xw = np.ascontiguousarray(x.transpose(2, 0, 1))  # [S, B, T]
    xw *= W1[:, None, :]
    xw = xw.astype(np.float16)
    W2T = W2.T.astype(ml_dtypes.bfloat16)  # [S_in, S_out]
    b1e = b1.astype(np.float32)
    in_maps = []
    for k in range(NCORES):
        sk = slice(k * SLOC, (k + 1) * SLOC)
        in_maps.append(
            {
                "xw": xw[sk],
                # [P, NSB] / [P, NSB, S] to match the SBUF tile dim order
                "b1e": np.ascontiguousarray(b1e[sk].reshape(NSB, P).T),
                "w2t": np.ascontiguousarray(
                    W2T[sk].reshape(NSB, P, S).transpose(1, 0, 2)
                ),
            }
        )
    return in_maps


def kernel(x, W1, b1, W2, b2):
    from concourse.bass_utils import run_bass_kernel_spmd

    nc = _get_nc()
    in_maps = _host_prep(
        np.asarray(x, dtype=np.float32),
        np.asarray(W1, dtype=np.float32),
        np.asarray(b1, dtype=np.float32),
        np.asarray(W2, dtype=np.float32),
    )
    res = run_bass_kernel_spmd(nc, in_maps, list(range(NCORES)))
    out = np.zeros((B, S), dtype=np.float32)
    for k in range(NCORES):
        out += res.results[k]["part"].astype(np.float32)
    out += np.asarray(b2, dtype=np.float32)[None, :]
    return out


# revision 16
# speedup vs baseline: 2.7676x; 1.1152x over previous
"""Trainium2 Bass kernel for nn_LogisticRegressionPerStock.

Math:  h = sigmoid(einsum("bts,st->bs", x, W1) + b1);  out = h @ W2.T + b2
Shapes: x [1024, 24, 8192], W1 [8192, 24], W2 [8192, 8192].

Sharding: stock dim S split across 8 cores (SLOC = 1024 each); each core
computes a full [B, S] partial of the final GEMM (contraction over its
local stocks); host sums the 8 bf16 partials and adds b2.

Per-core dataflow: W1 is folded into x on the host (x' = x * W1,
transposed to [S, B, T] fp16) so 128 stocks sit on SBUF partitions, t is
innermost, and the per-stock einsum is one DVE tensor_reduce (sum over t)
per (s-block, b-chunk); bias+sigmoid fuse into one ACT op writing hT
[s-part, b-free] bf16 — the GEMM's stationary layout, no PE transposes.
The [B,S] GEMM runs in bf16 (1 cycle/row), hT slices stationary (4
consecutive 512-wide matmuls per weight load), accumulating over the 8
local k-tiles in PSUM.  B is pipelined in chunks of 128: the reduce of
chunk c+1 overlaps the GEMM of chunk c, and within chunk 0 the GEMM's
k-accumulation consumes each s-block as its reduce lands.  Engine/queue
split: x + W2 loads on the SP queue (W2 streams as 8 column-pieces
behind the early x tiles); sigmoids, PSUM->SBUF copies and output stores
on the ACT engine/queue so stores never block the x stream.
"""

import sys

sys.path.insert(0, "/opt/trn_rl_repo")

import numpy as np
import ml_dtypes

B, T, S = 1024, 24, 8192
NCORES = 8
SLOC = S // NCORES  # 1024 stocks per core
P = 128
NSB = SLOC // P  # 8 s-blocks = GEMM k-tiles per core
BC = 128  # b-chunk size for the E/G pipeline
NBC = B // BC  # 8 chunks
NG = 4  # output column groups per chunk (2048 cols each)
GW = S // NG  # 2048
NPS = GW // 512  # 4 psum tiles per group

_compiled = {}


def _build_nc():
    import concourse.bass as bass
    import concourse.bacc as bacc
    import concourse.tile as tile
    from concourse import mybir

    f32 = mybir.dt.float32
    f16 = mybir.dt.float16
    bf16 = mybir.dt.bfloat16
    ADD = mybir.AluOpType.add

    nc = bacc.Bacc()
    # xw: [S_loc, B, T] fp16, xw[s, b, t] = x[b, t, s] * W1[s, t]
    xt_d = nc.dram_tensor("xw", [SLOC, B, T], f16, kind="ExternalInput")
    # b1e: [128, NSB]; [p, k] = b1[k*128+p]
    b1_d = nc.dram_tensor("b1e", [P, NSB], f32, kind="ExternalInput")
    # w2t: [128, NSB, S]; [p, k, n] = W2.T[k*128+p, n] for this core's rows
    w2_d = nc.dram_tensor("w2t", [P, NSB, S], bf16, kind="ExternalInput")
    out_d = nc.dram_tensor("part", [B, S], bf16, kind="ExternalOutput")

    with tile.TileContext(nc) as tc:
        with (
            tc.tile_pool(name="persist", bufs=1) as pp,
            # bufs=8: e_dma(c+1) issues all 8 x DMAs before e_compute(c+1)
            # is traced, so each DMA must land in a slot whose previous
            # occupant's reader (chunk c's reduce) is already traced
            tc.tile_pool(name="xp", bufs=8) as xp,
            tc.tile_pool(name="ep", bufs=4) as ep,
            tc.tile_pool(name="stp", bufs=2) as stp,
            tc.tile_pool(name="psp", bufs=8, space="PSUM") as psp,
        ):
            b1sb = pp.tile([P, NSB], f32, tag="b1sb")
            nc.sync.dma_start(b1sb[:], b1_d[:, :])
            w2sb = pp.tile([P, NSB, S], bf16, tag="w2sb")
            hts = [
                pp.tile([P, NSB, BC], bf16, name=f"ht{c}", tag=f"ht{c}")
                for c in range(NBC)
            ]
            xtiles = {}

            PW = S // NSB  # 1024-col W2 stream piece

            def e_dma(c):
                for sb in range(NSB):
                    xtile = xp.tile([P, BC, T], f16, tag="xt", name="xtile")
                    nc.sync.dma_start(
                        xtile[:],
                        xt_d[sb * P : (sb + 1) * P, c * BC : (c + 1) * BC, :],
                    )
                    xtiles[(c, sb)] = xtile
                    # W2 column-pieces stream behind the first x tiles; the
                    # GEMM's group-g matmuls block on their arrival via
                    # subtile RAW deps
                    if c == 0 and sb % 2 == 1:
                        for j in (sb - 1, sb):
                            nc.sync.dma_start(
                                w2sb[:, :, j * PW : (j + 1) * PW],
                                w2_d[:, :, j * PW : (j + 1) * PW],
                            )

            def e_compute(c):
                for sb in range(NSB):
                    acc = ep.tile([P, BC], f16, tag="acc")
                    with nc.allow_low_precision("fp16 sum of 24 terms, validated"):
                        nc.vector.tensor_reduce(
                            acc[:], xtiles[(c, sb)][:], mybir.AxisListType.X, ADD
                        )
                    nc.scalar.activation(
                        hts[c][:, sb, :],
                        acc[:],
                        mybir.ActivationFunctionType.Sigmoid,
                        bias=b1sb[:, sb : sb + 1],
                    )

            def phase_g(c):
                bt = c  # BC == P: one b-tile per chunk
                for g in range(NG):
                    pss = [
                        psp.tile([P, 512], f32, tag="ps", name=f"ps{n}")
                        for n in range(NPS)
                    ]
                    for k in range(NSB):
                        lhsT = hts[c][:, k, :]
                        for n in range(NPS):
                            nc.tensor.matmul(
                                pss[n][:],
                                lhsT,
                                w2sb[:, k, g * GW + n * 512 : g * GW + (n + 1) * 512],
                                start=(k == 0),
                                stop=(k == NSB - 1),
                            )
                    stg = stp.tile([P, GW], bf16, tag="stg", name="stg")
                    for n in range(NPS):
                        nc.scalar.activation(
                            stg[:, n * 512 : (n + 1) * 512],
                            pss[n][:],
                            mybir.ActivationFunctionType.Copy,
                        )
                    # out stores ride the ACT queue so they never block the
                    # x/W2 stream on the sync queue
                    nc.scalar.dma_start(
                        out_d[bt * P : (bt + 1) * P, g * GW : (g + 1) * GW],
                        stg[:],
                    )

            e_dma(0)
            e_compute(0)
            for c in range(NBC):
                if c + 1 < NBC:
                    e_dma(c + 1)
                phase_g(c)
                if c + 1 < NBC:
                    e_compute(c + 1)
    nc.finalize()
    return nc


def _get_nc():
    if "nc" not in _compiled:
        _compiled["nc"] = _build_nc()
    return _compiled["nc"]


def _host_prep(x, W1, b1, W2):
    # xw[s, b, t] = x[b, t, s] * W1[s, t], fp16
    xw = np.ascontiguousarray(x.transpose(2, 0, 1))  # [S, B, T]
    xw *= W1[:, None, :]
    xw = xw.astype(np.float16)
    W2T = W2.T.astype(ml_dtypes.bfloat16)  # [S_in, S_out]
    b1e = b1.astype(np.float32)
    in_maps = []
    for k in range(NCORES):
        sk = slice(k * SLOC, (k + 1) * SLOC)
        in_maps.append(
            {
                "xw": xw[sk],
                # [P, NSB] / [P, NSB, S] to match the SBUF tile dim order
                "b1e": np.ascontiguousarray(b1e[sk].reshape(NSB, P).T),
                "w2t": np.ascontiguousarray(
                    W2T[sk].reshape(NSB, P, S).transpose(1, 0, 2)
                ),
            }
        )
    return in_maps


def kernel(x, W1, b1, W2, b2):
    from concourse.bass_utils import run_bass_kernel_spmd

    nc = _get_nc()
    in_maps = _host_prep(
        np.asarray(x, dtype=np.float32),
        np.asarray(W1, dtype=np.float32),
        np.asarray(b1, dtype=np.float32),
        np.asarray(W2, dtype=np.float32),
    )
    res = run_bass_kernel_spmd(nc, in_maps, list(range(NCORES)))
    out = np.zeros((B, S), dtype=np.float32)
    for k in range(NCORES):
        out += res.results[k]["part"].astype(np.float32)
    out += np.asarray(b2, dtype=np.float32)[None, :]
    return out
